# revision 1
# baseline (speedup 1.0000x reference)
"""Trainium2 Bass kernel for BoxMultiHeadedAttention (B=4, S=1024, D=1024, H=16).

Reference math (eval mode, mask is all-ones so the masking is a no-op):
    qg/kg/qa/ka/va = per-head projections of the five inputs
    q = concat([qa, qg], -1); k = concat([ka, kg], -1)           # [B,H,S,128]
    p = softmax(q @ k.T / sqrt(128)); x = (p @ va) -> [B,S,D]
    out = sigmoid(concat([query_a, query_g], -1) @ Wgate.T + bgate) * (x @ Winfo.T + binfo)

Sharding: 8 cores = 4 batches x 2 head-halves. Core c handles batch c//2 and
heads (c%2)*8 .. +8 (which are also x-columns (c%2)*512..+512).  The GLU is
column-sharded the same way; the attention output halves are exchanged
between core pairs with a pairwise AllGather so each core can compute its
512 output columns of fc_info (which contracts over all 1024 x-dims).

Layout: everything is computed transposed ([feature, seq] with feature on
partitions) so projection biases and the softmax denominators line up with
per-partition scalars.  Scores are computed k-major (sT = k @ q.T), the
softmax denominator comes from a ones-column appended to va (row 64 of the
p@v accumulation), and the normalization happens after the p@v matmul on the
small [64, S] output.  bva folds into an adjusted fc_info bias on the host.

Precision: bf16 inputs/weights for projections, gate, p@v and fc_info;
float32r (full-speed 4-byte PE mode) for the q.k score matmul; fp32 psum
accumulation everywhere.
"""

import os

import ml_dtypes
import numpy as np

import concourse.bass as bass
import concourse.mybir as mybir
import concourse.tile as tile
from concourse import bacc, bass_utils

B, S, D, H = 4, 1024, 1024, 16
DK = D // H            # 64
CD = 2 * DK            # 128 concat head dim
HL = H // 2            # 8 local heads per core
T = D // 128           # 8 partition tiles per 1024 dim
NQ = S // 512          # 2 moving-dim blocks
SCALE = 1.0 / float(np.sqrt(2 * DK))

F32 = mybir.dt.float32
F32R = mybir.dt.float32r
BF16 = mybir.dt.bfloat16
NPBF16 = ml_dtypes.bfloat16

REPLICA_GROUPS = [[0, 1], [2, 3], [4, 5], [6, 7]]


def build_nc():
    nc = bacc.Bacc("TRN2", target_bir_lowering=False, debug=False, num_devices=8)

    # ---- DRAM I/O (per-core tensors; same program on all 8 cores) ----
    # big operands are laid out partition-major on the host ([128, T*n]) so
    # each DMA moves long contiguous per-partition lines at full HBM rate
    d_xqa = nc.dram_tensor("xqa", [128, T * S], BF16, kind="ExternalInput")
    d_xqg = nc.dram_tensor("xqg", [128, T * S], BF16, kind="ExternalInput")
    d_xka = nc.dram_tensor("xka", [128, T * S], BF16, kind="ExternalInput")
    d_xkg = nc.dram_tensor("xkg", [128, T * S], BF16, kind="ExternalInput")
    d_xv = nc.dram_tensor("xv", [128, T * S], BF16, kind="ExternalInput")
    d_wqa = nc.dram_tensor("wqa", [128, T * 512], BF16, kind="ExternalInput")
    d_wqg = nc.dram_tensor("wqg", [128, T * 512], BF16, kind="ExternalInput")
    d_wka = nc.dram_tensor("wka", [128, T * 512], BF16, kind="ExternalInput")
    d_wkg = nc.dram_tensor("wkg", [128, T * 512], BF16, kind="ExternalInput")
    d_wv = nc.dram_tensor("wv", [128, T * 512], BF16, kind="ExternalInput")
    d_wg = nc.dram_tensor("wg", [128, 2 * T * 512], BF16, kind="ExternalInput")
    d_wi = nc.dram_tensor("wi", [128, T * 512], BF16, kind="ExternalInput")
    d_bq = nc.dram_tensor("bq", [CD, HL], F32, kind="ExternalInput")
    d_bk = nc.dram_tensor("bk", [CD, HL], F32, kind="ExternalInput")
    d_bg = nc.dram_tensor("bg", [128, 4], F32, kind="ExternalInput")
    d_bi = nc.dram_tensor("bi", [128, 4], F32, kind="ExternalInput")
    d_out = nc.dram_tensor("out", [4, 128, S], F32, kind="ExternalOutput")

    with tile.TileContext(nc) as tc:
        with (
            tc.tile_pool(name="xin", bufs=1) as p_xin,
            tc.tile_pool(name="wts", bufs=1) as p_w,
            tc.tile_pool(name="big", bufs=1) as p_big,
            tc.tile_pool(name="att", bufs=1) as p_att,
            tc.tile_pool(name="tail", bufs=1) as p_tail,
            tc.tile_pool(name="psA", bufs=1, space="PSUM") as p_psA,
            tc.tile_pool(name="psS", bufs=1, space="PSUM") as p_psS,
            tc.tile_pool(name="psX", bufs=1, space="PSUM") as p_psX,
            tc.tile_pool(name="dram", bufs=1, space="DRAM") as p_dram,
        ):
            # --- persistent sbuf tiles (tags control slot reuse) ---
            t_xv = p_xin.tile([128, T, S], BF16, tag="vin", bufs=1)
            t_xqa = p_xin.tile([128, T, S], BF16, tag="qin", bufs=2)
            t_xqg = p_xin.tile([128, T, S], BF16, tag="qin", bufs=2)
            t_xka = p_xin.tile([128, T, S], BF16, tag="kin", bufs=2)
            t_xkg = p_xin.tile([128, T, S], BF16, tag="kin", bufs=2)

            t_wv = p_w.tile([128, T, 512], BF16, tag="w8", bufs=3)
            t_wqa = p_w.tile([128, T, 512], BF16, tag="w8", bufs=3)
            t_wqg = p_w.tile([128, T, 512], BF16, tag="w8", bufs=3)
            t_wka = p_w.tile([128, T, 512], BF16, tag="w8", bufs=3)
            t_wkg = p_w.tile([128, T, 512], BF16, tag="w8", bufs=3)
            t_wi = p_w.tile([128, T, 512], BF16, tag="w8", bufs=3)

            t_bq = p_w.tile([CD, HL], F32, tag="bias", bufs=4)
            t_bk = p_w.tile([CD, HL], F32, tag="bias", bufs=4)
            t_bg = p_w.tile([128, 4], F32, tag="bias", bufs=4)
            t_bi = p_w.tile([128, 4], F32, tag="bias", bufs=4)

            t_qT = p_big.tile([128, HL, S], F32R, tag="b32", bufs=2)
            t_kT = p_big.tile([128, HL, S], F32R, tag="b32", bufs=2)

            t_va = p_att.tile([128, T, HL, DK + 1], BF16, tag="va", bufs=1)
            t_xt = p_xin.tile([128, 4, S], BF16, tag="kin", bufs=2)

            # --- PE warmup: keep TensorE busy during the DMA lead-in so HAM
            # un-throttles before the real matmuls arrive ---
            t_wu = p_att.tile([128, 512], BF16, tag="wu", bufs=1)
            nc.vector.memset(t_wu[:], 0.0)
            for _ in range(24):
                pwu = p_psA.tile([128, 512], F32, tag="proj", bufs=2)
                nc.tensor.matmul(pwu[:], t_wu[:, 0:128], t_wu[:],
                                 start=True, stop=True)

            # --- load inputs / weights, in consumption order; 2-tile
            # chunks so several DMA queues run in parallel ---
            def load(dt_, tl, n_t, chunk=2):
                r = dt_.ap().rearrange("p (t n) -> p t n", t=n_t)
                for tt in range(0, n_t, chunk):
                    nc.sync.dma_start(tl[:, tt:tt + chunk, :],
                                      r[:, tt:tt + chunk, :])

            load(d_wv, t_wv, T)
            load(d_xv, t_xv, T)
            load(d_wqa, t_wqa, T)
            load(d_wqg, t_wqg, T)
            load(d_xqa, t_xqa, T)
            load(d_xqg, t_xqg, T)
            load(d_wka, t_wka, T)
            load(d_wkg, t_wkg, T)
            load(d_xka, t_xka, T)
            load(d_xkg, t_xkg, T)
            load(d_wi, t_wi, T)
            nc.sync.dma_start(t_bq[:], d_bq.ap())
            nc.sync.dma_start(t_bk[:], d_bk.ap())
            nc.sync.dma_start(t_bg[:], d_bg.ap())
            nc.sync.dma_start(t_bi[:], d_bi.ap())

            # --- va projection (natural [s, dk] layout, + ones column) ---
            nc.vector.memset(t_va[:, :, :, DK:DK + 1], 1.0)
            for st in range(T):
                ps = p_psA.tile([128, 512], F32, tag="proj", bufs=2)
                for kt in range(T):
                    nc.tensor.matmul(
                        ps[:],
                        t_xv[:, kt, st * 128:(st + 1) * 128],
                        t_wv[:, kt, :],
                        start=(kt == 0), stop=(kt == T - 1),
                    )
                nc.vector.tensor_copy(
                    t_va[:, st, :, 0:DK],
                    ps[:].rearrange("p (h d) -> p h d", h=HL),
                )

            # filler matmuls: bridge the DMA window between the va and
            # q projections so the PE never idles long enough to re-throttle.
            # Reading t_va pins them after the va projection in the schedule.
            for _ in range(76):
                pwu = p_psA.tile([128, 512], F32, tag="proj", bufs=2,
                                 name="pwu_fill")
                nc.tensor.matmul(pwu[0:DK + 1, :], t_va[:, 0, 0, :], t_wu[:],
                                 start=True, stop=True)

            # --- q / k projections (transposed, concat layout) ---
            # psum rows 0:64 <- qa-head dims (weights col-tile 0), rows 64:128
            # <- qg-head dims (col-tile 64); the two M=64 matmuls per step run
            # concurrently in distinct PE column groups.
            for (wa, wb, xa, xb, dst, bias) in [
                (t_wqa, t_wqg, t_xqa, t_xqg, t_qT, t_bq),
                (t_wka, t_wkg, t_xka, t_xkg, t_kT, t_bk),
            ]:
                for h in range(HL):
                    for n in range(NQ):
                        ps = p_psA.tile([128, 512], F32, tag="proj", bufs=2)
                        for kt in range(T):
                            nc.tensor.matmul(
                                ps[0:64, :],
                                wa[:, kt, h * DK:(h + 1) * DK],
                                xa[:, kt, n * 512:(n + 1) * 512],
                                start=(kt == 0), stop=(kt == T - 1),
                                tile_position=(0, 0), skip_group_check=True,
                            )
                            nc.tensor.matmul(
                                ps[64:128, :],
                                wb[:, kt, h * DK:(h + 1) * DK],
                                xb[:, kt, n * 512:(n + 1) * 512],
                                start=(kt == 0), stop=(kt == T - 1),
                                tile_position=(0, 64), skip_group_check=True,
                            )
                        nc.vector.tensor_scalar_add(
                            dst[:, h, n * 512:(n + 1) * 512], ps[:],
                            bias[:, h:h + 1],
                        )

            # --- attention (k-major scores, flash-style over k tiles) ---
            def attn_head(h):
                px = p_psX.tile([DK + 1, S], F32, tag="x", bufs=2,
                                name=f"px_{h}")
                for kt in range(T):
                    te = p_att.tile([128, S], BF16, tag="exp", bufs=3,
                                    name=f"te_{h}_{kt}")
                    for n in range(NQ):
                        pss = p_psS.tile([128, 512], F32, tag="s", bufs=2,
                                         name=f"pss_{h}_{kt}_{n}")
                        nc.tensor.matmul(
                            pss[:],
                            t_kT[:, h, kt * 128:(kt + 1) * 128],
                            t_qT[:, h, n * 512:(n + 1) * 512],
                            start=True, stop=True,
                        )
                        nc.scalar.activation(
                            te[:, n * 512:(n + 1) * 512], pss[:],
                            mybir.ActivationFunctionType.Exp, scale=SCALE,
                        )
                    for n in range(NQ):
                        nc.tensor.matmul(
                            px[:, n * 512:(n + 1) * 512],
                            t_va[:, kt, h, :],
                            te[:, n * 512:(n + 1) * 512],
                            start=(kt == 0), stop=(kt == T - 1),
                        )
                # normalize: row DK of px holds the softmax denominator.
                # 1/denom = exp(-ln(denom)) right behind this head's exps on
                # ACT, so the division (and the x-block exchange behind it)
                # never trails the head.  (nc.vector.reciprocal costs ~6.5us
                # on a 1-partition operand and made every ship late.)
                t_ln = p_att.tile([1, S], F32, tag="recip", bufs=1,
                                  name=f"ln_{h}")
                nc.scalar.activation(t_ln[:], px[DK:DK + 1, :],
                                     mybir.ActivationFunctionType.Ln)
                t_recip = p_att.tile([1, S], F32, tag="recip2", bufs=1,
                                     name=f"recip_{h}")
                nc.scalar.activation(t_recip[:], t_ln[:],
                                     mybir.ActivationFunctionType.Exp,
                                     scale=-1.0)
                t_bc = p_att.tile([DK, S], F32, tag="bc", bufs=1,
                                  name=f"bc_{h}")
                nc.gpsimd.partition_broadcast(t_bc[:], t_recip[:])
                nc.vector.tensor_tensor(
                    t_xt[(h % 2) * DK:(h % 2) * DK + DK, h // 2, :],
                    px[0:DK, :], t_bc[:], op=mybir.AluOpType.mult,
                )

            # Ship each 2-head x block with its own pairwise AllGather as
            # soon as it completes: blocks 0-2 hide entirely under the rest
            # of attention; block 3's exchange is covered by gate + the
            # early info accumulation below.  Local block i holds my x-dims
            # i*128..; the gather adds the partner's dims (4+i)*128-aligned.
            t_xtf = p_xin.tile([128, T, S], BF16, tag="kin", bufs=2)

            def ship_block(i):
                cc_in = p_dram.tile([1, 128, S], BF16, name=f"cci_{i}")
                cc_out = p_dram.tile([2, 128, S], BF16, name=f"cco_{i}")
                nc.sync.dma_start(cc_in[0], t_xt[:, i, :])
                nc.gpsimd.collective_compute(
                    "AllGather", mybir.AluOpType.bypass,
                    replica_groups=REPLICA_GROUPS,
                    ins=[cc_in[:].opt()], outs=[cc_out[:].opt()],
                )
                nc.sync.dma_start(t_xtf[:, i, :], cc_out[0])
                nc.sync.dma_start(t_xtf[:, 4 + i, :], cc_out[1])

            for blk in range(4):
                attn_head(2 * blk)
                attn_head(2 * blk + 1)
                ship_block(blk)

            # --- gate (fills PE idle in the ACT-bound attention phase and
            # the final exchange wait) ---
            t_wg = p_xin.tile([128, 2 * T, 512], BF16, tag="vin", bufs=1)
            load(d_wg, t_wg, 2 * T)
            t_gate = p_big.tile([128, 4, S], BF16, tag="gate", bufs=1)
            for mt in range(4):
                for n in range(NQ):
                    ps = p_psA.tile([128, 512], F32, tag="proj", bufs=2)
                    for kt in range(2 * T):
                        xsrc = t_xqa if kt < T else t_xqg
                        nc.tensor.matmul(
                            ps[:],
                            t_wg[:, kt, mt * 128:(mt + 1) * 128],
                            xsrc[:, kt % T, n * 512:(n + 1) * 512],
                            start=(kt == 0), stop=(kt == 2 * T - 1),
                        )
                    nc.scalar.activation(
                        t_gate[:, mt, n * 512:(n + 1) * 512], ps[:],
                        mybir.ActivationFunctionType.Sigmoid,
                        bias=t_bg[:, mt:mt + 1],
                    )

            # --- info + GLU product + store ---
            # contract x-dims in exchange-arrival order so most of the
            # accumulation can run before the last AllGather lands
            info_kts = (0, 4, 1, 5, 2, 6, 3, 7)
            for mt in range(4):
                for n in range(NQ):
                    ps = p_psA.tile([128, 512], F32, tag="proj", bufs=2)
                    for i, kt in enumerate(info_kts):
                        nc.tensor.matmul(
                            ps[:],
                            t_wi[:, kt, mt * 128:(mt + 1) * 128],
                            t_xtf[:, kt, n * 512:(n + 1) * 512],
                            start=(i == 0), stop=(i == T - 1),
                        )
                    t_ob = p_tail.tile([128, 512], F32, tag="outb", bufs=2)
                    nc.vector.scalar_tensor_tensor(
                        t_ob[:], ps[:], t_bi[:, mt:mt + 1],
                        t_gate[:, mt, n * 512:(n + 1) * 512],
                        op0=mybir.AluOpType.add, op1=mybir.AluOpType.mult,
                    )
                    nc.sync.dma_start(
                        d_out.ap()[mt, :, n * 512:(n + 1) * 512], t_ob[:])

    nc.compile()
    return nc


def make_in_maps(inputs):
    """Host-side sharding: transpose/slice/cast the full inputs per core."""
    f32 = np.float32
    g = {k: np.asarray(v) for k, v in inputs.items()}
    binfo_eff = (
        g["binfo"].astype(np.float64)
        + g["Winfo"].astype(np.float64) @ g["bva"].astype(np.float64)
    ).astype(f32)

    in_maps = []
    for c in range(8):
        b, hh = c // 2, c % 2
        hs = slice(hh * 512, (hh + 1) * 512)

        def pmajor(a):
            # [1024*k, n] -> partition-major [128, k*T*n]-style layout the
            # kernel DMAs as long contiguous per-partition lines
            rows, n = a.shape
            t = rows // 128
            return np.ascontiguousarray(
                a.reshape(t, 128, n).transpose(1, 0, 2).reshape(128, t * n))

        def xt(name):
            return pmajor(g[name][b].T.astype(NPBF16))

        def wt(name):
            return pmajor(g[name][hs].T.astype(NPBF16))

        def bqk(pa, pg):
            a = g[pa][hs].reshape(HL, DK).T.astype(f32)   # [64, 8]
            gg = g[pg][hs].reshape(HL, DK).T.astype(f32)
            return np.ascontiguousarray(np.vstack([a, gg]))  # [128, 8]

        m = {
            "xqa": xt("query_a"), "xqg": xt("query_g"),
            "xka": xt("key_a"), "xkg": xt("key_g"), "xv": xt("value_a"),
            "wqa": wt("Wqa"), "wqg": wt("Wqg"),
            "wka": wt("Wka"), "wkg": wt("Wkg"), "wv": wt("Wva"),
            "wg": wt("Wgate"), "wi": wt("Winfo"),
            "bq": bqk("bqa", "bqg"), "bk": bqk("bka", "bkg"),
            "bg": np.ascontiguousarray(
                g["bgate"][hs].reshape(4, 128).T.astype(f32)),
            "bi": np.ascontiguousarray(
                binfo_eff[hs].reshape(4, 128).T.astype(f32)),
        }
        in_maps.append(m)
    return in_maps


def assemble(results):
    out = np.empty((B, S, D), dtype=np.float32)
    for c in range(8):
        b, hh = c // 2, c % 2
        blk = results[c]["out"].reshape(512, S)   # [cols, seq]
        out[b, :, hh * 512:(hh + 1) * 512] = blk.T
    return out


_NC_CACHE = {}


def _get_nc():
    if "nc" not in _NC_CACHE:
        _NC_CACHE["nc"] = build_nc()
    return _NC_CACHE["nc"]


LAST_RESULTS = None


def kernel(**inputs) -> np.ndarray:
    global LAST_RESULTS
    nc = _get_nc()
    in_maps = make_in_maps(inputs)
    trace = os.environ.get("KERNEL_TRACE", "0") == "1"
    kwargs = {}
    if trace:
        kwargs["trace_cores"] = list(range(8))
    res = bass_utils.run_bass_kernel_spmd(
        nc, in_maps, core_ids=list(range(8)), trace=trace, **kwargs,
    )
    LAST_RESULTS = res
    return assemble(res.results)



# revision 12
# speedup vs baseline: 1.1417x; 1.1417x over previous
"""Trainium2 Bass kernel for BoxMultiHeadedAttention (B=4, S=1024, D=1024, H=16).

Reference math (eval mode, mask is all-ones so the masking is a no-op):
    qg/kg/qa/ka/va = per-head projections of the five inputs
    q = concat([qa, qg], -1); k = concat([ka, kg], -1)           # [B,H,S,128]
    p = softmax(q @ k.T / sqrt(128)); x = (p @ va) -> [B,S,D]
    out = sigmoid(concat([query_a, query_g], -1) @ Wgate.T + bgate) * (x @ Winfo.T + binfo)

Sharding: 8 cores = 4 batches x 2 head-halves. Core c handles batch c//2 and
heads (c%2)*8 .. +8 (which are also x-columns (c%2)*512..+512).  The GLU is
column-sharded the same way; attention-output halves are exchanged between
core pairs with per-2-head-block pairwise AllGathers.

v2 schedule (vs the v1 baseline):
  * scores in bf16 (v1 used fp32 "HIGH" mode matmuls at ~2.7x the cost)
  * exp batched over [128,1024] PSUM reads (one ACT op per (head, key-tile))
  * phase order: q/k projections -> ACT-paced attention with va-projection and
    gate matmuls textually interleaved as PE fillers -> info tail.
  * info contraction is split: my own x blocks are consumed straight out of
    SBUF (no collective round trip) against host-permuted Winfo tiles; the
    partner half flows through the AllGathers.  Both gather rows are always
    contracted (12 tiles total); the row that echoes this core's own data gets
    zero weights from the host, which keeps the program parity-free.
  * px is copied out of PSUM right after the last p@v so the single px bank
    pair recycles immediately instead of blocking on the recip/normalize chain.

Layout: compute transposed ([feature, seq] on partitions); scores k-major
(sT = k @ q.T); softmax denominator from a ones-column in va (row 64 of the
p@v output); bva folds into the fc_info bias host-side.
"""

import os

import ml_dtypes
import numpy as np

import concourse.bass as bass
import concourse.mybir as mybir
import concourse.tile as tile
from concourse import bacc, bass_utils

B, S, D, H = 4, 1024, 1024, 16
DK = D // H            # 64
CD = 2 * DK            # 128 concat head dim
HL = H // 2            # 8 local heads per core
T = D // 128           # 8 partition tiles per 1024 dim
NQ = S // 512          # 2 moving-dim blocks
SCALE = 1.0 / float(np.sqrt(2 * DK))

F32 = mybir.dt.float32
BF16 = mybir.dt.bfloat16
NPBF16 = ml_dtypes.bfloat16

REPLICA_GROUPS = [[0, 1], [2, 3], [4, 5], [6, 7]]


def build_nc():
    nc = bacc.Bacc("TRN2", target_bir_lowering=False, debug=False, num_devices=8)

    # ---- DRAM I/O (per-core tensors; same program on all 8 cores) ----
    d_xqa = nc.dram_tensor("xqa", [128, T * S], BF16, kind="ExternalInput")
    d_xqg = nc.dram_tensor("xqg", [128, T * S], BF16, kind="ExternalInput")
    d_xka = nc.dram_tensor("xka", [128, T * S], BF16, kind="ExternalInput")
    d_xkg = nc.dram_tensor("xkg", [128, T * S], BF16, kind="ExternalInput")
    d_xv = nc.dram_tensor("xv", [128, T * S], BF16, kind="ExternalInput")
    d_wqa = nc.dram_tensor("wqa", [128, T * 512], BF16, kind="ExternalInput")
    d_wqg = nc.dram_tensor("wqg", [128, T * 512], BF16, kind="ExternalInput")
    d_wka = nc.dram_tensor("wka", [128, T * 512], BF16, kind="ExternalInput")
    d_wkg = nc.dram_tensor("wkg", [128, T * 512], BF16, kind="ExternalInput")
    d_wv = nc.dram_tensor("wv", [128, T * 512], BF16, kind="ExternalInput")
    d_wg = nc.dram_tensor("wg", [128, 2 * T * 512], BF16, kind="ExternalInput")
    # 12 contraction tiles: 0-3 my x blocks, 4-11 the two gather rows per block
    d_wi = nc.dram_tensor("wi", [128, 12 * 512], BF16, kind="ExternalInput")
    d_bq = nc.dram_tensor("bq", [CD, HL], F32, kind="ExternalInput")
    d_bk = nc.dram_tensor("bk", [CD, HL], F32, kind="ExternalInput")
    d_bg = nc.dram_tensor("bg", [128, 4], F32, kind="ExternalInput")
    d_bi = nc.dram_tensor("bi", [128, 4], F32, kind="ExternalInput")
    d_out = nc.dram_tensor("out", [4, 128, S], F32, kind="ExternalOutput")

    with tile.TileContext(nc) as tc:
        with (
            tc.tile_pool(name="xin", bufs=1) as p_xin,
            tc.tile_pool(name="wts", bufs=1) as p_w,
            tc.tile_pool(name="big", bufs=1) as p_big,
            tc.tile_pool(name="att", bufs=1) as p_att,
            tc.tile_pool(name="tail", bufs=1) as p_tail,
            tc.tile_pool(name="psA", bufs=1, space="PSUM") as p_psA,
            tc.tile_pool(name="psS", bufs=1, space="PSUM") as p_psS,
            tc.tile_pool(name="psX", bufs=1, space="PSUM") as p_psX,
            tc.tile_pool(name="dram", bufs=1, space="DRAM") as p_dram,
        ):
            # --- persistent sbuf tiles ---
            t_xqa = p_xin.tile([128, T, S], BF16, tag="qin", bufs=2)
            t_xqg = p_xin.tile([128, T, S], BF16, tag="qin", bufs=2)
            t_xka = p_xin.tile([128, T, S], BF16, tag="kin", bufs=2)
            t_xkg = p_xin.tile([128, T, S], BF16, tag="kin", bufs=2)
            t_xv = p_xin.tile([128, T, S], BF16, tag="vin", bufs=1)

            t_wqa = p_w.tile([128, T, 512], BF16, tag="w8", bufs=4)
            t_wqg = p_w.tile([128, T, 512], BF16, tag="w8", bufs=4)
            t_wka = p_w.tile([128, T, 512], BF16, tag="w8", bufs=4)
            t_wkg = p_w.tile([128, T, 512], BF16, tag="w8", bufs=4)
            t_wv = p_w.tile([128, T, 512], BF16, tag="w8", bufs=4)
            t_wi = p_w.tile([128, 12, 512], BF16, tag="wi", bufs=1)

            t_bq = p_w.tile([CD, HL], F32, tag="bias", bufs=4)
            t_bk = p_w.tile([CD, HL], F32, tag="bias", bufs=4)
            t_bg = p_w.tile([128, 4], F32, tag="bias", bufs=4)
            t_bi = p_w.tile([128, 4], F32, tag="bias", bufs=4)

            t_qT = p_big.tile([128, HL, S], BF16, tag="qk", bufs=2)
            t_kT = p_big.tile([128, HL, S], BF16, tag="qk", bufs=2)

            t_va = p_att.tile([128, T, HL, DK + 1], BF16, tag="va", bufs=1)
            # my normalized x blocks [xdim, block, seq]
            t_xt = p_xin.tile([128, 4, S], BF16, tag="kin", bufs=2)
            # gathered remote rows: [xdim, block, row(2), seq]
            t_xr = p_xin.tile([128, 4, 2, S], BF16, tag="kin", bufs=2)
            # gate output
            t_gate = p_big.tile([128, 4, S], BF16, tag="gate", bufs=1)
            # info partial sums (local half, bias folded in)
            t_part = p_big.tile([128, 4, NQ, 512], BF16, tag="part", bufs=1)

            # --- load inputs / weights in consumption order ---
            def load(dt_, tl, n_t, chunk=2):
                r = dt_.ap().rearrange("p (t n) -> p t n", t=n_t)
                for tt in range(0, n_t, chunk):
                    nc.sync.dma_start(tl[:, tt:tt + chunk, :],
                                      r[:, tt:tt + chunk, :])

            load(d_wqa, t_wqa, T)
            load(d_wqg, t_wqg, T)
            load(d_xqa, t_xqa, T)
            load(d_xqg, t_xqg, T)
            load(d_wka, t_wka, T)
            load(d_wkg, t_wkg, T)
            load(d_xka, t_xka, T)
            load(d_xkg, t_xkg, T)
            nc.sync.dma_start(t_bq[:], d_bq.ap())
            nc.sync.dma_start(t_bk[:], d_bk.ap())
            nc.sync.dma_start(t_bg[:], d_bg.ap())
            nc.sync.dma_start(t_bi[:], d_bi.ap())
            load(d_wv, t_wv, T)
            load(d_xv, t_xv, T)
            load(d_wi, t_wi, 12)

            # --- PE warmup: a short burst so HAM un-throttles during the
            # DMA lead-in ---
            t_wu = p_att.tile([128, 512], BF16, tag="wu", bufs=1)
            nc.vector.memset(t_wu[:], 0.0)
            for _ in range(12):
                pwu = p_psA.tile([128, 512], F32, tag="proj", bufs=2)
                nc.tensor.matmul(pwu[:], t_wu[:, 0:128], t_wu[:],
                                 start=True, stop=True)

            nc.vector.memset(t_va[:, :, :, DK:DK + 1], 1.0)

            # --- q/k projections (transposed concat layout, concurrent
            # M=64 column-group pairs) ---
            for h in range(HL):
                for (wa, wb, xa, xb, dst, bias) in [
                    (t_wqa, t_wqg, t_xqa, t_xqg, t_qT, t_bq),
                    (t_wka, t_wkg, t_xka, t_xkg, t_kT, t_bk),
                ]:
                    for n in range(NQ):
                        ps = p_psA.tile([128, 512], F32, tag="proj", bufs=2)
                        for kt in range(T):
                            nc.tensor.matmul(
                                ps[0:64, :],
                                wa[:, kt, h * DK:(h + 1) * DK],
                                xa[:, kt, n * 512:(n + 1) * 512],
                                start=(kt == 0), stop=(kt == T - 1),
                                tile_position=(0, 0), skip_group_check=True,
                            )
                            nc.tensor.matmul(
                                ps[64:128, :],
                                wb[:, kt, h * DK:(h + 1) * DK],
                                xb[:, kt, n * 512:(n + 1) * 512],
                                start=(kt == 0), stop=(kt == T - 1),
                                tile_position=(0, 64), skip_group_check=True,
                            )
                        nc.vector.tensor_scalar_add(
                            dst[:, h, n * 512:(n + 1) * 512], ps[:],
                            bias[:, h:h + 1],
                        )

            # ---------- PE filler units for the ACT-paced attention ----------
            # Each unit emits one psum group of matmuls + its epilogue.
            # Supply: 8 va-projection groups, 8 gate groups, and the per-block
            # local-info partial groups are emitted at block boundaries.
            filler_units = []

            def va_unit(st):
                def emit():
                    ps = p_psA.tile([128, 512], F32, tag="proj", bufs=2)
                    for kt in range(T):
                        nc.tensor.matmul(
                            ps[:],
                            t_xv[:, kt, st * 128:(st + 1) * 128],
                            t_wv[:, kt, :],
                            start=(kt == 0), stop=(kt == T - 1),
                        )
                    nc.vector.tensor_copy(
                        t_va[:, st, :, 0:DK],
                        ps[:].rearrange("p (h d) -> p h d", h=HL),
                    )
                return emit

            # gate weight tiles reuse the xv slot once va is done; load lazily
            t_wg = p_xin.tile([128, 2 * T, 512], BF16, tag="vin", bufs=1)
            _wg_loaded = [False]

            def gate_unit(mt, n):
                def emit():
                    if not _wg_loaded[0]:
                        _wg_loaded[0] = True
                        load(d_wg, t_wg, 2 * T)
                    ps = p_psA.tile([128, 512], F32, tag="proj", bufs=2)
                    for kt in range(2 * T):
                        xsrc = t_xqa if kt < T else t_xqg
                        nc.tensor.matmul(
                            ps[:],
                            t_wg[:, kt, mt * 128:(mt + 1) * 128],
                            xsrc[:, kt % T, n * 512:(n + 1) * 512],
                            start=(kt == 0), stop=(kt == 2 * T - 1),
                        )
                    nc.scalar.activation(
                        t_gate[:, mt, n * 512:(n + 1) * 512], ps[:],
                        mybir.ActivationFunctionType.Sigmoid,
                        bias=t_bg[:, mt:mt + 1],
                    )
                return emit

            # first 6 gate groups fill attention; last 2 cover the cc tail
            # (va units are emitted inside attention head 0, just in time)
            gate_order = [(mt, n) for mt in range(4) for n in range(NQ)]
            for mt, n in gate_order[:6]:
                filler_units.append(gate_unit(mt, n))
            _fill_pos = [0]

            def emit_fillers(k):
                while k > 0 and _fill_pos[0] < len(filler_units):
                    filler_units[_fill_pos[0]]()
                    _fill_pos[0] += 1
                    k -= 1

            # local-info partials: one matmul + one DVE accumulate per
            # (block, mt, n).  Bias folded in at block 0.
            def local_info_block(b):
                for mt in range(4):
                    for n in range(NQ):
                        ps = p_psA.tile([128, 512], F32, tag="proj", bufs=2)
                        nc.tensor.matmul(
                            ps[:],
                            t_wi[:, b, mt * 128:(mt + 1) * 128],
                            t_xt[:, b, n * 512:(n + 1) * 512],
                            start=True, stop=True,
                        )
                        if b == 0:
                            nc.vector.tensor_scalar_add(
                                t_part[:, mt, n, :], ps[:],
                                t_bi[:, mt:mt + 1],
                            )
                        else:
                            nc.vector.tensor_tensor(
                                t_part[:, mt, n, :], ps[:],
                                t_part[:, mt, n, :], op=mybir.AluOpType.add,
                            )

            # --- attention (k-major scores, flash over key tiles) ---
            def attn_head(h):
                px = p_psX.tile([DK + 1, S], F32, tag="x", bufs=1,
                                name=f"px_{h}")
                tes = []
                pv_done = [0]

                def pv(j):
                    for n in range(NQ):
                        nc.tensor.matmul(
                            px[:, n * 512:(n + 1) * 512],
                            t_va[:, j, h, :],
                            tes[j][:, n * 512:(n + 1) * 512],
                            start=(j == 0), stop=(j == T - 1),
                        )
                    pv_done[0] = j + 1

                for kt in range(T):
                    pss = p_psS.tile([128, S], F32, tag="s", bufs=2,
                                     name=f"pss_{h}_{kt}")
                    for n in range(NQ):
                        nc.tensor.matmul(
                            pss[:, n * 512:(n + 1) * 512],
                            t_kT[:, h, kt * 128:(kt + 1) * 128],
                            t_qT[:, h, n * 512:(n + 1) * 512],
                            start=True, stop=True,
                        )
                    te = p_att.tile([128, S], BF16, tag="exp", bufs=3,
                                    name=f"te_{h}_{kt}")
                    nc.scalar.activation(
                        te[:], pss[:],
                        mybir.ActivationFunctionType.Exp, scale=SCALE,
                    )
                    tes.append(te)
                    if h == 0:
                        # va projection tiles are produced here, just in time:
                        # p@v for key-tile j must follow va tile j in the PE
                        # queue, so head 0 runs p@v at a lag of 2.
                        if kt == 2:
                            va_unit(0)(); va_unit(1)(); va_unit(2)()
                            pv(0); pv(1)
                        elif kt >= 3:
                            va_unit(kt)()
                            pv(kt - 1)
                    else:
                        if kt > 0:
                            pv(kt - 1)
                        if kt in (2, 5):
                            emit_fillers(1)
                while pv_done[0] < T:
                    pv(pv_done[0])
                # copy px out of PSUM immediately so its bank pair recycles
                t_px = p_att.tile([DK + 1, S], BF16, tag="pxs", bufs=2,
                                  name=f"pxs_{h}")
                nc.vector.tensor_copy(t_px[:], px[:])
                # 1/denom = exp(-ln(denom)), then broadcast and normalize
                t_ln = p_att.tile([1, S], F32, tag="recip", bufs=1,
                                  name=f"ln_{h}")
                nc.scalar.activation(t_ln[:], t_px[DK:DK + 1, :],
                                     mybir.ActivationFunctionType.Ln)
                t_recip = p_att.tile([1, S], F32, tag="recip2", bufs=1,
                                     name=f"recip_{h}")
                nc.scalar.activation(t_recip[:], t_ln[:],
                                     mybir.ActivationFunctionType.Exp,
                                     scale=-1.0)
                t_bc = p_att.tile([DK, S], F32, tag="bc", bufs=1,
                                  name=f"bc_{h}")
                nc.gpsimd.partition_broadcast(t_bc[:], t_recip[:])
                nc.vector.tensor_tensor(
                    t_xt[(h % 2) * DK:(h % 2) * DK + DK, h // 2, :],
                    t_px[0:DK, :], t_bc[:], op=mybir.AluOpType.mult,
                )

            def ship_block(i):
                cc_in = p_dram.tile([1, 128, S], BF16, name=f"cci_{i}")
                cc_out = p_dram.tile([2, 128, S], BF16, name=f"cco_{i}")
                nc.sync.dma_start(cc_in[0], t_xt[:, i, :])
                nc.gpsimd.collective_compute(
                    "AllGather", mybir.AluOpType.bypass,
                    replica_groups=REPLICA_GROUPS,
                    ins=[cc_in[:].opt()], outs=[cc_out[:].opt()],
                )
                nc.sync.dma_start(t_xr[:, i, 0, :], cc_out[0])
                nc.sync.dma_start(t_xr[:, i, 1, :], cc_out[1])

            for blk in range(4):
                attn_head(2 * blk)
                attn_head(2 * blk + 1)
                ship_block(blk)
                local_info_block(blk)

            # --- tail: remaining gate + remote info + GLU product + store ---
            for mt, n in gate_order[6:]:
                gate_unit(mt, n)()

            for mt in range(4):
                for n in range(NQ):
                    ps = p_psA.tile([128, 512], F32, tag="proj", bufs=2)
                    k = 0
                    for j in range(4):
                        for r in range(2):
                            nc.tensor.matmul(
                                ps[:],
                                t_wi[:, 4 + 2 * j + r, mt * 128:(mt + 1) * 128],
                                t_xr[:, j, r, n * 512:(n + 1) * 512],
                                start=(k == 0), stop=(k == 7),
                            )
                            k += 1
                    t_ob = p_tail.tile([128, 512], F32, tag="outb", bufs=2)
                    nc.vector.tensor_tensor(
                        t_ob[:], ps[:], t_part[:, mt, n, :],
                        op=mybir.AluOpType.add,
                    )
                    nc.vector.tensor_tensor(
                        t_ob[:], t_ob[:],
                        t_gate[:, mt, n * 512:(n + 1) * 512],
                        op=mybir.AluOpType.mult,
                    )
                    nc.sync.dma_start(
                        d_out.ap()[mt, :, n * 512:(n + 1) * 512], t_ob[:])

    nc.compile()
    return nc


def make_in_maps(inputs):
    """Host-side sharding: transpose/slice/cast the full inputs per core."""
    f32 = np.float32
    g = {k: np.asarray(v) for k, v in inputs.items()}
    binfo_eff = (
        g["binfo"].astype(np.float64)
        + g["Winfo"].astype(np.float64) @ g["bva"].astype(np.float64)
    ).astype(f32)

    in_maps = []
    for c in range(8):
        b, hh = c // 2, c % 2
        hs = slice(hh * 512, (hh + 1) * 512)

        def pmajor(a):
            rows, n = a.shape
            t = rows // 128
            return np.ascontiguousarray(
                a.reshape(t, 128, n).transpose(1, 0, 2).reshape(128, t * n))

        def xt(name):
            return pmajor(g[name][b].T.astype(NPBF16))

        def wt(name):
            return pmajor(g[name][hs].T.astype(NPBF16))

        def bqk(pa, pg):
            a = g[pa][hs].reshape(HL, DK).T.astype(f32)   # [64, 8]
            gg = g[pg][hs].reshape(HL, DK).T.astype(f32)
            return np.ascontiguousarray(np.vstack([a, gg]))  # [128, 8]

        # Winfo contraction tiles, permuted per core:
        #   tiles 0-3  : my own x-dim blocks (global tile hh*4+j)
        #   tiles 4-11 : gather rows -- tile 4+2j+r is row r (core parity r)
        #                of block j = global x-tile r*4+j; zero when r == hh
        #                (that row echoes this core's own data).
        wiT = g["Winfo"][hs].T.astype(np.float64)   # [1024 xdims, 512 outs]
        wi_tiles = []
        for j in range(4):
            gt = hh * 4 + j
            wi_tiles.append(wiT[gt * 128:(gt + 1) * 128, :])
        for j in range(4):
            for r in range(2):
                if r == hh:
                    wi_tiles.append(np.zeros((128, 512)))
                else:
                    gt = r * 4 + j
                    wi_tiles.append(wiT[gt * 128:(gt + 1) * 128, :])
        wi = np.ascontiguousarray(
            np.concatenate([t[None] for t in wi_tiles], axis=0)  # [12,128,512]
            .transpose(1, 0, 2).reshape(128, 12 * 512).astype(NPBF16))

        m = {
            "xqa": xt("query_a"), "xqg": xt("query_g"),
            "xka": xt("key_a"), "xkg": xt("key_g"), "xv": xt("value_a"),
            "wqa": wt("Wqa"), "wqg": wt("Wqg"),
            "wka": wt("Wka"), "wkg": wt("Wkg"), "wv": wt("Wva"),
            "wg": wt("Wgate"), "wi": wi,
            "bq": bqk("bqa", "bqg"), "bk": bqk("bka", "bkg"),
            "bg": np.ascontiguousarray(
                g["bgate"][hs].reshape(4, 128).T.astype(f32)),
            "bi": np.ascontiguousarray(
                binfo_eff[hs].reshape(4, 128).T.astype(f32)),
        }
        in_maps.append(m)
    return in_maps


def assemble(results):
    out = np.empty((B, S, D), dtype=np.float32)
    for c in range(8):
        b, hh = c // 2, c % 2
        blk = results[c]["out"].reshape(512, S)   # [cols, seq]
        out[b, :, hh * 512:(hh + 1) * 512] = blk.T
    return out


_NC_CACHE = {}


def _get_nc():
    if "nc" not in _NC_CACHE:
        _NC_CACHE["nc"] = build_nc()
    return _NC_CACHE["nc"]


LAST_RESULTS = None


def kernel(**inputs) -> np.ndarray:
    global LAST_RESULTS
    nc = _get_nc()
    in_maps = make_in_maps(inputs)
    trace = os.environ.get("KERNEL_TRACE", "0") == "1"
    kwargs = {}
    if trace:
        kwargs["trace_cores"] = list(range(8))
    res = bass_utils.run_bass_kernel_spmd(
        nc, in_maps, core_ids=list(range(8)), trace=trace, **kwargs,
    )
    LAST_RESULTS = res
    return assemble(res.results)
